# revision 14
# baseline (speedup 1.0000x reference)
"""ContactMapHead Trainium2 kernel (v12: fp8 X + W-hi/lo DoubleRow proj).

Reference computation (per batch b):
    h = relu(X @ W^T + pb)            # [S, DP]
    scores = (h @ h^T) * cw + cb      # [S, S]  -- symmetric!

Sharding over 8 NeuronCores: core c handles batch b = c//2 with roll
offset off = (c%2)*1024 applied to X on the host. Each core computes
hT = relu(W @ XT + pb) for its full (rolled) batch, then emits the
circulant band of the symmetric score map: local tile rows i_t in 0..7
(tiles of 128), local cols j_t in i_t..i_t+8 (9 tiles of 128). Across
the two cores of a batch pair plus host-side transpose mirroring this
covers all 16x16 global tiles exactly.

v12 design (measured facts driving it):
- Every HWDGE queue tops out ~135GB/s and ROUND-ROBINS its queued
  DMAs (fair by bytes): pre-queuing all chunks makes them complete
  together at the end. Fix: strict per-queue order + quarter-size
  sub-DMAs so the 3-semaphore issue window keeps chunks ~in order.
- fp8 matmuls run at 1.0 cycle/column (same as bf16); DoubleRow's
  benefit is the 2-wide slot dim. Here the slots carry (Whi, Wlo) -
  a two-term fp8 expansion of W*64 - against X broadcast with a
  stride-0 slot AP (measured exact). X is then a SINGLE fp8 copy:
  input drops from 4.7MB (bf16) to 2.6MB, removing all input stalls.
  Cost: X fp8 quantization error -> rel err 1.43e-2 (tol 2e-2),
  bit-exact with the host numpy simulation of the same pipeline.
- Scale bookkeeping: psum = 64*(X@W^T), relu bias = 64*pb, final
  multiplier cw/4096 (relu is positively homogeneous).
- Tiny-descriptor DMAs (pb/cwb) stay on the SWDGE ring: on a HWDGE
  ring their per-descriptor cost throttles the whole queue.
- PE warm-up matmuls trip the HAM duty monitor during the input wait.
"""

import numpy as np
import ml_dtypes

from concourse import bacc, masks, mybir, tile

BF = ml_dtypes.bfloat16
F8 = ml_dtypes.float8_e4m3

P = 128
B, S, D = 4, 2048, 1024
DP = 256  # projection dim
NCORES = 8
KT = D // P  # 8 k-tiles over D
PT = DP // P  # 2 p-tiles over DP
CHK = 512  # input DMA chunk width (s columns)
NCH = S // CHK  # 4 chunks
NROW = 8  # local band rows (tiles of 128) per core
BANDW = 9 * P  # 1152 band columns per row
SEG = BANDW // 3  # 384-col band chunks
NWARM = 9
SW = 64.0  # power-of-2 scale folded into Wq

f32 = mybir.dt.float32
bf16 = mybir.dt.bfloat16
fp8 = mybir.dt.float8e4
DR = mybir.MatmulPerfMode.DoubleRow


def _build_nc():
    nc = bacc.Bacc()
    xt = nc.declare_dram_parameter("xt", [P, KT, S], fp8, isOutput=False)
    wt = nc.declare_dram_parameter("wt", [P, KT, 2, DP], fp8, isOutput=False)
    pb = nc.declare_dram_parameter("pb", [DP], f32, isOutput=False)
    cwb = nc.declare_dram_parameter("cwb", [2], f32, isOutput=False)
    out = nc.declare_dram_parameter("out", [NROW, P, BANDW], bf16, isOutput=True)

    with tile.TileContext(nc) as tc:
        _body(nc, tc, xt, wt, pb, cwb, out)
    nc.compile()
    return nc


def _body(nc, tc, xt, wt, pb, cwb, out):
    mult = mybir.AluOpType.mult
    add = mybir.AluOpType.add
    Relu = mybir.ActivationFunctionType.Relu
    Ident = mybir.ActivationFunctionType.Identity

    with (
        tc.tile_pool(name="const", bufs=1) as cpool,
        tc.tile_pool(name="orow", bufs=NROW) as opool,
        tc.tile_pool(name="pj", bufs=3, space="PSUM") as pj,
        tc.tile_pool(name="pw", bufs=5, space="PSUM") as pw,
    ):
        # ---- PE warm-up while the input streams.
        ident = cpool.tile([P, P], f32, tag="ident")
        masks.make_identity(nc, ident[:])
        wps = pw.tile([P, SEG], f32, tag="pw", name="warm")
        for _ in range(NWARM):
            nc.tensor.matmul(wps[:, 0:P], ident[:], ident[:], start=True, stop=True)

        # ---- tiny constants on the SWDGE ring.
        pb_t = cpool.tile([P, PT], f32, tag="pb_t")
        nc.gpsimd.dma_start(pb_t[:], pb.ap().rearrange("(t p) -> p t", p=P))

        cwb_t = cpool.tile([P, 2], f32, tag="cwb_t")
        nc.gpsimd.dma_start(cwb_t[:], cwb.ap().partition_broadcast(P))

        # ---- x chunk 0 leads, then wt (k-halves, whole-partition, one
        # per queue), then the remaining chunks - strictly ordered on
        # both HWDGE queues, in partition-quarter sub-DMAs so the
        # per-engine DMA semaphore window keeps completion near
        # in-order (a queue round-robins fairly among in-flight DMAs).
        wt_t = cpool.tile([P, KT, 2, DP], fp8, tag="wt_t")
        xtile = cpool.tile([P, KT, S], fp8, tag="xtile")
        xv = xt.ap()

        def quarters(e0, e1, emit):
            emit(e0, 0, 32)
            emit(e0, 32, 64)
            emit(e1, 64, 96)
            emit(e1, 96, 128)

        def xchunk(ch):
            c0 = ch * CHK
            quarters(
                nc.sync, nc.scalar,
                lambda e, p0, p1: e.dma_start(
                    xtile[p0:p1, :, c0 : c0 + CHK], xv[p0:p1, :, c0 : c0 + CHK]
                ),
            )

        xchunk(0)
        quarters(
            nc.sync, nc.scalar,
            lambda e, p0, p1: e.dma_start(wt_t[p0:p1], wt.ap()[p0:p1]),
        )
        for ch in range(1, NCH):
            xchunk(ch)

        # hT for the whole local map; relu writes per (pt, chunk) slices
        ht = cpool.tile([P, PT, S], bf16, tag="ht")

        def project(c0, c1):
            w = c1 - c0
            for pt in range(PT):
                pjs = pj.tile([P, 512], f32, tag="pj", name="pj")
                for k in range(KT):
                    nc.tensor.matmul(
                        pjs[:, 0:w],
                        wt_t[:, k, :, pt * P : (pt + 1) * P],
                        xtile[:, k, c0:c1].unsqueeze(1).broadcast_to([P, 2, w]),
                        start=(k == 0),
                        stop=(k == KT - 1),
                        perf_mode=DR,
                    )
                # psum = 64*(X@W^T)+err; relu(psum + 64*pb) = 64*h.
                if pt == 0:
                    nc.scalar.activation(
                        ht[:, pt, c0:c1],
                        pjs[:, 0:w],
                        Relu,
                        bias=pb_t[:, pt : pt + 1],
                    )
                else:
                    nc.vector.tensor_scalar(
                        ht[:, pt, c0:c1],
                        pjs[:, 0:w],
                        pb_t[:, pt : pt + 1],
                        0.0,
                        add,
                        mybir.AluOpType.max,
                    )

        def emit_pair_row(i_t):
            """Band row i_t: out[i_t] = cw' * hT_i^T @ hT[band cols] + cb."""
            base = i_t * P
            psums = []
            for pt in range(PT):
                for si in range(3):
                    if pt == 0:
                        psums.append(pw.tile([P, SEG], f32, tag="pw", name="pw"))
                    c0 = base + si * SEG
                    nc.tensor.matmul(
                        psums[si][:],
                        ht[:, pt, base : base + P],
                        ht[:, pt, c0 : c0 + SEG],
                        start=(pt == 0),
                        stop=(pt == PT - 1),
                    )
            orow = opool.tile([P, BANDW], bf16, tag="orow", name="orow")
            tail = i_t >= NROW - 2
            for si in range(3):
                dst = orow[:, si * SEG : (si + 1) * SEG]
                if tail:
                    # last rows: per-segment drains, alternating engines
                    k = i_t * 3 + si
                    if k % 2 == 0:
                        nc.vector.tensor_scalar(
                            dst, psums[si][:], cwb_t[:, 0:1], cwb_t[:, 1:2],
                            mult, add,
                        )
                    else:
                        nc.scalar.activation(
                            dst, psums[si][:], Ident,
                            bias=cwb_t[:, 1:2], scale=cwb_t[:, 0:1],
                        )
                    nc.sync.dma_start(
                        out.ap()[i_t][:, si * SEG : (si + 1) * SEG], dst
                    )
                elif (i_t * 3 + si) % 2 == 0:
                    nc.vector.tensor_scalar(
                        dst, psums[si][:], cwb_t[:, 0:1], cwb_t[:, 1:2], mult, add
                    )
                else:
                    nc.scalar.activation(
                        dst, psums[si][:], Ident,
                        bias=cwb_t[:, 1:2], scale=cwb_t[:, 0:1],
                    )
            if not tail:
                # mid-stream rows ride the HWDGE queues (input is done
                # by then); keeping the SWDGE queue empty of outputs
                # makes the end-of-kernel drain barrier fast.
                eng = nc.sync if i_t % 2 == 0 else nc.scalar
                eng.dma_start(out.ap()[i_t], orow[:])

        # dovetail: rows 0-3 need h cols < 1536 (chunks 0-2); rows 4-7
        # need the full projection.
        project(0, 512)
        project(512, 1024)
        project(1024, 1536)
        for i_t in range(4):
            emit_pair_row(i_t)
        project(1536, 2048)
        for i_t in range(4, NROW):
            emit_pair_row(i_t)


_NC_CACHE = None


def _get_nc():
    global _NC_CACHE
    if _NC_CACHE is None:
        _NC_CACHE = _build_nc()
    return _NC_CACHE


def _pack_pks(mat_T, rows, cols, dt):
    """[rows*P, cols] -> [P, rows, cols] with d = k*P + p split as (k, p)."""
    return np.ascontiguousarray(
        mat_T.astype(dt).reshape(rows, P, cols).transpose(1, 0, 2)
    )


def _make_in_maps(hidden_states, proj_w, proj_b, clf_w, clf_b):
    hs = np.asarray(hidden_states, dtype=np.float32)
    wv = np.asarray(proj_w, dtype=np.float32)
    pbv = np.ascontiguousarray(
        np.asarray(proj_b, dtype=np.float32).reshape(DP) * SW
    )
    cw = float(np.asarray(clf_w).reshape(-1)[0])
    cb = float(np.asarray(clf_b).reshape(-1)[0])
    cwbv = np.array([cw / (SW * SW), cb], dtype=np.float32)

    # W*64 as an (hi, lo) fp8 pair: wt[p, k, s, m] with d = k*P + p
    ws = (wv * SW).T  # [D, DP] f32
    whi = ws.astype(F8)
    wlo = (ws - whi.astype(np.float32)).astype(F8)
    wtv = np.empty((P, KT, 2, DP), dtype=F8)
    wtv[:, :, 0, :] = whi.reshape(KT, P, DP).transpose(1, 0, 2)
    wtv[:, :, 1, :] = wlo.reshape(KT, P, DP).transpose(1, 0, 2)

    in_maps = []
    for b in range(B):
        xpks = _pack_pks(hs[b].T, KT, S, F8)  # [P, KT, S] fp8
        xpks_r = np.ascontiguousarray(np.roll(xpks, -S // 2, axis=2))
        for xv_ in (xpks, xpks_r):
            in_maps.append({"xt": xv_, "wt": wtv, "pb": pbv, "cwb": cwbv})
    return in_maps


def _assemble(results):
    scores = np.empty((B, S, S), np.float32)
    for c in range(NCORES):
        b, half = divmod(c, 2)
        o = np.asarray(results[c]["out"], dtype=np.float32)  # [NROW, P, BANDW]
        for i_t in range(NROW):
            gi = i_t + NROW * half
            strip = o[i_t]
            for lj in range(i_t, i_t + 9):
                gj = (lj + NROW * half) % 16
                V = strip[:, (lj - i_t) * P : (lj - i_t + 1) * P]
                scores[b, gi * P : (gi + 1) * P, gj * P : (gj + 1) * P] = V
                if gj != gi:
                    scores[b, gj * P : (gj + 1) * P, gi * P : (gi + 1) * P] = V.T
    return scores


def kernel(hidden_states, proj_w, proj_b, clf_w, clf_b):
    from concourse.bass_utils import run_bass_kernel_spmd

    nc = _get_nc()
    in_maps = _make_in_maps(hidden_states, proj_w, proj_b, clf_w, clf_b)
    res = run_bass_kernel_spmd(nc, in_maps, core_ids=list(range(NCORES)))
    return _assemble(res.results)


def run_traced(hidden_states, proj_w, proj_b, clf_w, clf_b):
    """Like kernel(), but also returns BassKernelResults with trace info."""
    from concourse.bass_utils import run_bass_kernel_spmd

    nc = _get_nc()
    in_maps = _make_in_maps(hidden_states, proj_w, proj_b, clf_w, clf_b)
    res = run_bass_kernel_spmd(
        nc, in_maps, core_ids=list(range(NCORES)), trace=True
    )
    return _assemble(res.results), res
